# revision 4
# baseline (speedup 1.0000x reference)
"""BPMLL loss kernel for Trainium2, 8-core data parallel.

Reference computation (B=128, L=1024):
    y[b,i]     = target[b,i] == 1
    truth      = y[:,:,None] & ~y[:,None,:]
    inner[b]   = sum_{i,j} truth * exp(x[b,j] - x[b,i])
    length[b]  = n_pos[b] * n_neg[b]
    out        = sum_b inner[b] / length[b]

The O(L^2) pairwise sum factorizes:
    inner[b] = (sum_{j: ~y} exp(x[b,j])) * (sum_{i: y} exp(-x[b,i]))

Sharding: pure data parallel, batch split across 8 cores (16 samples each).
Each core's [16, 1024] slice is laid out as [128, 128] in SBUF (8 partition
rows per sample).  Masking is folded into the exp argument:
    t1 = x - 100*y
    exp(t1)        = (1-y)*exp(x)   + O(e^-96)   -> row-sum via ACT accum
    exp(-t1 - 100) = y*exp(-x)      + O(e^-96)   -> row-sum via ACT accum
A [128,16] segment matrix matmul reduces the 8 rows per sample, then a short
per-sample tail computes inner/length.  The host sums the 128 per-sample
losses (the all-reduce/gather step of the data-parallel scheme).
"""

import os
import sys

import numpy as np

if "/opt/trn_rl_repo" not in sys.path:
    sys.path.insert(0, "/opt/trn_rl_repo")

from contextlib import ExitStack

import concourse.bass as bass  # noqa: F401  (import keeps bass registered)
import concourse.tile as tile
from concourse import bacc, mybir
from concourse.bass_utils import run_bass_kernel_spmd


def _ensure_ntff_hook():
    """Provide antenv.axon_hooks if the image lacks it, so trace=True /
    BASS_TRACE=1 profiling works instead of crashing on import."""
    import types

    try:
        from antenv.axon_hooks import get_axon_ntff_profile_hook  # noqa: F401

        return
    except ImportError:
        pass
    try:
        import antenv
    except ImportError:
        return
    mod = types.ModuleType("antenv.axon_hooks")
    mod._hook = None

    def set_axon_ntff_profile_hook(h):
        mod._hook = h

    def get_axon_ntff_profile_hook():
        return mod._hook

    mod.set_axon_ntff_profile_hook = set_axon_ntff_profile_hook
    mod.get_axon_ntff_profile_hook = get_axon_ntff_profile_hook
    sys.modules["antenv.axon_hooks"] = mod
    antenv.axon_hooks = mod
    try:
        from trn_agent_boot.trn_boot import _ntff_profile_via_ctypes

        hook = _ntff_profile_via_ctypes("/opt/axon/libaxon_pjrt.so")
        if hook is not None:
            mod._hook = hook
    except Exception:
        pass


_ensure_ntff_hook()

B, L = 128, 1024
NCORES = 8
BS = B // NCORES            # 16 samples per core
P = 128                     # SBUF partitions
F = (BS * L) // P           # 128 free elements per partition row
RPS = P // BS               # 8 partition rows per sample
MASK_BIG = 100.0            # exp(-96) ~ 2e-42: kills masked terms in f32 sums

_cached_nc = None


def _build_module():
    nc = bacc.Bacc(
        "TRN2",
        target_bir_lowering=False,
        debug=False,
        num_devices=NCORES,
    )
    xs_d = nc.dram_tensor("xs", [P, F], mybir.dt.float32, kind="ExternalInput").ap()
    ys_d = nc.dram_tensor("ys", [P, F], mybir.dt.uint8, kind="ExternalInput").ap()
    seg_d = nc.dram_tensor("seg", [P, BS], mybir.dt.float32, kind="ExternalInput").ap()
    out_d = nc.dram_tensor("out", [BS, 1], mybir.dt.float32, kind="ExternalOutput").ap()

    with tile.TileContext(nc) as tc:
        with ExitStack() as ctx:
            pool = ctx.enter_context(tc.tile_pool(name="main", bufs=1))
            psum = ctx.enter_context(tc.tile_pool(name="psum", bufs=1, space="PSUM"))

            x_t = pool.tile([P, F], mybir.dt.float32)
            nc.sync.dma_start(x_t[:], xs_d)
            y_t = pool.tile([P, F], mybir.dt.uint8)
            nc.sync.dma_start(y_t[:], ys_d)
            seg_t = pool.tile([P, BS], mybir.dt.float32)
            nc.sync.dma_start(seg_t[:], seg_d)

            # stats[:,0] = row sum (1-y)exp(x); [:,1] = row sum y*exp(-x);
            # [:,2] = row n_pos
            stats = pool.tile([P, 3], mybir.dt.float32)

            nbig = pool.tile([P, 1], mybir.dt.float32)
            nc.gpsimd.memset(nbig[:], -MASK_BIG)

            t1 = pool.tile([P, F], mybir.dt.float32)
            nc.vector.scalar_tensor_tensor(
                t1[:],
                y_t[:],
                -MASK_BIG,
                x_t[:],
                op0=mybir.AluOpType.mult,
                op1=mybir.AluOpType.add,
            )
            nc.vector.reduce_sum(stats[:, 2:3], y_t[:], axis=mybir.AxisListType.X)

            e1 = pool.tile([P, F], mybir.dt.float32)
            nc.scalar.activation(
                e1[:],
                t1[:],
                mybir.ActivationFunctionType.Exp,
                accum_out=stats[:, 0:1],
            )
            e2 = pool.tile([P, F], mybir.dt.float32)
            nc.scalar.activation(
                e2[:],
                t1[:],
                mybir.ActivationFunctionType.Exp,
                bias=nbig[:],
                scale=-1.0,
                accum_out=stats[:, 1:2],
            )

            # Segment-reduce the 8 rows per sample: [16,3] = seg.T @ stats
            ps = psum.tile([BS, 3], mybir.dt.float32)
            nc.tensor.matmul(ps[:], seg_t[:], stats[:])

            # Per-sample tail: loss = S_neg*S_pos / (n_pos*(L-n_pos))
            sps = pool.tile([BS, 3], mybir.dt.float32)
            nc.scalar.copy(sps[:], ps[:])

            inner = pool.tile([BS, 1], mybir.dt.float32)
            nc.vector.tensor_mul(inner[:], sps[:, 0:1], sps[:, 1:2])
            nneg = pool.tile([BS, 1], mybir.dt.float32)
            nc.vector.tensor_scalar(
                nneg[:],
                sps[:, 2:3],
                -1.0,
                float(L),
                op0=mybir.AluOpType.mult,
                op1=mybir.AluOpType.add,
            )
            length = pool.tile([BS, 1], mybir.dt.float32)
            nc.vector.tensor_mul(length[:], sps[:, 2:3], nneg[:])
            rlen = pool.tile([BS, 1], mybir.dt.float32)
            nc.vector.reciprocal(rlen[:], length[:])
            loss = pool.tile([BS, 1], mybir.dt.float32)
            nc.vector.tensor_mul(loss[:], inner[:], rlen[:])

            nc.sync.dma_start(out_d, loss[:])

    nc.compile()
    return nc


def get_module():
    global _cached_nc
    if _cached_nc is None:
        _cached_nc = _build_module()
    return _cached_nc


def _make_seg() -> np.ndarray:
    seg = np.zeros((P, BS), dtype=np.float32)
    seg[np.arange(P), np.arange(P) // RPS] = 1.0
    return seg


def make_in_maps(input: np.ndarray, target: np.ndarray) -> list[dict]:
    x = np.ascontiguousarray(input, dtype=np.float32)
    y = np.ascontiguousarray((target != 0).astype(np.uint8))
    seg = _make_seg()
    in_maps = []
    for c in range(NCORES):
        in_maps.append(
            {
                "xs": x[c * BS : (c + 1) * BS].reshape(P, F),
                "ys": y[c * BS : (c + 1) * BS].reshape(P, F),
                "seg": seg,
            }
        )
    return in_maps


def kernel(input: np.ndarray, target: np.ndarray) -> np.ndarray:
    assert input.shape == (B, L) and target.shape == (B, L)
    nc = get_module()
    in_maps = make_in_maps(np.asarray(input), np.asarray(target))
    res = run_bass_kernel_spmd(nc, in_maps, core_ids=list(range(NCORES)))
    losses = np.concatenate([np.asarray(r["out"]).reshape(BS) for r in res.results])
    return np.asarray(losses.sum(), dtype=np.float32)


# revision 6
# speedup vs baseline: 1.2190x; 1.2190x over previous
"""BPMLL loss kernel for Trainium2, 8-core data parallel.

Reference computation (B=128, L=1024):
    y[b,i]     = target[b,i] == 1
    truth      = y[:,:,None] & ~y[:,None,:]
    inner[b]   = sum_{i,j} truth * exp(x[b,j] - x[b,i])
    length[b]  = n_pos[b] * n_neg[b]
    out        = sum_b inner[b] / length[b]

The O(L^2) pairwise sum factorizes:
    inner[b] = (sum_{j: ~y} exp(x[b,j])) * (sum_{i: y} exp(-x[b,i]))

Sharding: pure data parallel, batch split across 8 cores (16 samples each).
Each core's [16, 1024] slice is laid out as [128, 128] in SBUF (8 partition
rows per sample).  Masking is folded into the exp argument:
    t1 = x - 100*y
    exp(t1)        = (1-y)*exp(x)   + O(e^-96)   -> row-sum via ACT accum
    exp(-t1 - 100) = y*exp(-x)      + O(e^-96)   -> row-sum via ACT accum
A [128,16] segment matrix matmul reduces the 8 rows per sample, then a short
per-sample tail computes inner/length.  The host sums the 128 per-sample
losses (the all-reduce/gather step of the data-parallel scheme).
"""

import os
import sys

import numpy as np

if "/opt/trn_rl_repo" not in sys.path:
    sys.path.insert(0, "/opt/trn_rl_repo")

from contextlib import ExitStack

import concourse.bass as bass  # noqa: F401  (import keeps bass registered)
import concourse.tile as tile
from concourse import bacc, mybir
from concourse.bass_utils import run_bass_kernel_spmd


def _ensure_ntff_hook():
    """Provide antenv.axon_hooks if the image lacks it, so trace=True /
    BASS_TRACE=1 profiling works instead of crashing on import."""
    import types

    try:
        from antenv.axon_hooks import get_axon_ntff_profile_hook  # noqa: F401

        return
    except ImportError:
        pass
    try:
        import antenv
    except ImportError:
        return
    mod = types.ModuleType("antenv.axon_hooks")
    mod._hook = None

    def set_axon_ntff_profile_hook(h):
        mod._hook = h

    def get_axon_ntff_profile_hook():
        return mod._hook

    mod.set_axon_ntff_profile_hook = set_axon_ntff_profile_hook
    mod.get_axon_ntff_profile_hook = get_axon_ntff_profile_hook
    sys.modules["antenv.axon_hooks"] = mod
    antenv.axon_hooks = mod
    try:
        from trn_agent_boot.trn_boot import _ntff_profile_via_ctypes

        hook = _ntff_profile_via_ctypes("/opt/axon/libaxon_pjrt.so")
        if hook is not None:
            mod._hook = hook
    except Exception:
        pass


_ensure_ntff_hook()

B, L = 128, 1024
NCORES = 8
BS = B // NCORES            # 16 samples per core
P = 128                     # SBUF partitions
F = (BS * L) // P           # 128 free elements per partition row
RPS = P // BS               # 8 partition rows per sample
MASK_BIG = 100.0            # exp(-96) ~ 2e-42: kills masked terms in f32 sums

_cached_nc = None


def _build_module():
    nc = bacc.Bacc(
        "TRN2",
        target_bir_lowering=False,
        debug=False,
        num_devices=NCORES,
    )
    # One packed input per core: per partition row, 512B of x (f32), 128B of
    # y (u8 mask), 64B of seg (f32) -> one 704B/partition DMA.
    BLOB = F * 4 + F + BS * 4
    blob_d = nc.dram_tensor(
        "blob", [P, BLOB], mybir.dt.uint8, kind="ExternalInput"
    ).ap()
    out_d = nc.dram_tensor("out", [BS, 1], mybir.dt.float32, kind="ExternalOutput").ap()

    with tile.TileContext(nc) as tc:
        with ExitStack() as ctx:
            pool = ctx.enter_context(tc.tile_pool(name="main", bufs=1))
            psum = ctx.enter_context(tc.tile_pool(name="psum", bufs=1, space="PSUM"))

            blob = pool.tile([P, BLOB], mybir.dt.uint8)
            nc.sync.dma_start(blob[:], blob_d)
            x_t = blob[:, 0 : F * 4].bitcast(mybir.dt.float32)
            y_t = blob[:, F * 4 : F * 4 + F]
            seg_t = blob[:, F * 4 + F : BLOB].bitcast(mybir.dt.float32)

            # stats[:,0] = row sum (1-y)exp(x); [:,1] = row sum y*exp(-x);
            # [:,2] = row n_pos
            stats = pool.tile([P, 3], mybir.dt.float32)

            nbig = pool.tile([P, 1], mybir.dt.float32)
            nc.gpsimd.memset(nbig[:], -MASK_BIG)

            t1 = pool.tile([P, F], mybir.dt.float32)
            nc.vector.scalar_tensor_tensor(
                t1[:],
                y_t[:],
                -MASK_BIG,
                x_t[:],
                op0=mybir.AluOpType.mult,
                op1=mybir.AluOpType.add,
            )
            nc.vector.reduce_sum(stats[:, 2:3], y_t[:], axis=mybir.AxisListType.X)

            e1 = pool.tile([P, F], mybir.dt.float32)
            nc.scalar.activation(
                e1[:],
                t1[:],
                mybir.ActivationFunctionType.Exp,
                accum_out=stats[:, 0:1],
            )
            e2 = pool.tile([P, F], mybir.dt.float32)
            nc.scalar.activation(
                e2[:],
                t1[:],
                mybir.ActivationFunctionType.Exp,
                bias=nbig[:],
                scale=-1.0,
                accum_out=stats[:, 1:2],
            )

            # Segment-reduce the 8 rows per sample: [16,3] = seg.T @ stats
            ps = psum.tile([BS, 3], mybir.dt.float32)
            nc.tensor.matmul(ps[:], seg_t[:], stats[:])

            # Per-sample tail: loss = S_neg*S_pos / (n_pos*(L-n_pos))
            # (each DVE op reads at most one PSUM operand)
            nneg = pool.tile([BS, 1], mybir.dt.float32)
            nc.vector.tensor_scalar(
                nneg[:],
                ps[:, 2:3],
                -1.0,
                float(L),
                op0=mybir.AluOpType.mult,
                op1=mybir.AluOpType.add,
            )
            length = pool.tile([BS, 1], mybir.dt.float32)
            nc.vector.tensor_mul(length[:], ps[:, 2:3], nneg[:])
            rlen = pool.tile([BS, 1], mybir.dt.float32)
            nc.vector.reciprocal(rlen[:], length[:])
            u = pool.tile([BS, 1], mybir.dt.float32)
            nc.vector.tensor_mul(u[:], ps[:, 0:1], rlen[:])
            loss = pool.tile([BS, 1], mybir.dt.float32)
            nc.vector.tensor_mul(loss[:], ps[:, 1:2], u[:])

            nc.sync.dma_start(out_d, loss[:])

    nc.compile()
    return nc


def get_module():
    global _cached_nc
    if _cached_nc is None:
        _cached_nc = _build_module()
    return _cached_nc


def _make_seg() -> np.ndarray:
    seg = np.zeros((P, BS), dtype=np.float32)
    seg[np.arange(P), np.arange(P) // RPS] = 1.0
    return seg


def make_in_maps(input: np.ndarray, target: np.ndarray) -> list[dict]:
    x = np.ascontiguousarray(input, dtype=np.float32)
    y = np.ascontiguousarray((target != 0).astype(np.uint8))
    seg8 = _make_seg().view(np.uint8)  # [P, BS*4]
    in_maps = []
    for c in range(NCORES):
        xs8 = x[c * BS : (c + 1) * BS].reshape(P, F).view(np.uint8)  # [P, F*4]
        ys8 = y[c * BS : (c + 1) * BS].reshape(P, F)  # [P, F]
        blob = np.concatenate([xs8, ys8, seg8], axis=1)  # [P, 704] u8
        in_maps.append({"blob": blob})
    return in_maps


def kernel(input: np.ndarray, target: np.ndarray) -> np.ndarray:
    assert input.shape == (B, L) and target.shape == (B, L)
    nc = get_module()
    in_maps = make_in_maps(np.asarray(input), np.asarray(target))
    res = run_bass_kernel_spmd(nc, in_maps, core_ids=list(range(NCORES)))
    losses = np.concatenate([np.asarray(r["out"]).reshape(BS) for r in res.results])
    return np.asarray(losses.sum(), dtype=np.float32)
